# revision 23
# baseline (speedup 1.0000x reference)
"""Trainium2 Bass kernel for nn_AdjAttenAggr (masked attention aggregation).

Math (reference):
    Q = main_feat @ Wq.T + bq                 [N, MID]
    K = other_feat @ Wk.T + bk                [M, MID]
    A = softmax(where(mask, -2^32, Q K^T / sqrt(MID)), axis=-1)
    out = A @ (fix_feat[:, None] * other_feat)   [N, KDIM]

Strategy: row-parallel across 8 NeuronCores (1024 rows of N each), K/V
replicated.  On each core, flash-style: S^T tiles [m=128, q] from PE
(bf16 operands, fp32 PSUM), exp on ACT (scale folded in; scores are
small so no max subtraction is needed), multiplicative mask on DVE
(exp(s+NEG*mask) == exp(s)*(1-mask) since exp(-2^32) == 0), then
P^T @ V on PE accumulating over m.  Row sums ride along as a
ones-stationary matmul; normalization happens after the AV matmul since
softmax(S) @ V == diag(1/rowsum) @ (exp(S) @ V).
"""

import numpy as np
import ml_dtypes

import concourse.bacc as bacc
import concourse.tile as tile
from concourse import mybir
from concourse.bass_utils import run_bass_kernel_spmd
from concourse.masks import make_identity

BF16 = ml_dtypes.bfloat16

N, M = 8192, 8192
QDIM, KDIM, MID = 512, 512, 256
SCALE = float(np.sqrt(MID))
NCORES = 8
NSH = N // NCORES          # 1024 rows per core
P = 128                    # partitions
N_MT = M // P              # 64 m-tiles
N_ST = NSH // 512          # 2 supertiles of 512 q rows
N_JT = QDIM // P           # 4 contraction tiles for projections
N_DT = MID // P            # 2 d-tiles
G = 4                      # m-tiles per group
N_G = N_MT // G            # 16 groups

_BUILT = None


def build_nc():
    nc = bacc.Bacc(None, target_bir_lowering=False, debug=False)
    f32 = mybir.dt.float32
    bf = mybir.dt.bfloat16

    mainT = nc.declare_dram_parameter("mainT", [QDIM, NSH], bf, isOutput=False)
    otherT = nc.declare_dram_parameter("otherT", [KDIM, M], bf, isOutput=False)
    other = nc.declare_dram_parameter("other", [M, KDIM], bf, isOutput=False)
    nmaskT = nc.declare_dram_parameter("nmaskT", [M, NSH], bf, isOutput=False)
    wqT = nc.declare_dram_parameter("wqT", [QDIM, MID], bf, isOutput=False)
    wkT = nc.declare_dram_parameter("wkT", [KDIM, MID], bf, isOutput=False)
    bq_t = nc.declare_dram_parameter("bq_t", [P, N_DT], f32, isOutput=False)
    bk_t = nc.declare_dram_parameter("bk_t", [P, N_DT], f32, isOutput=False)
    fix_t = nc.declare_dram_parameter("fix_t", [P, N_MT], f32, isOutput=False)
    out = nc.declare_dram_parameter("out", [NSH, KDIM], f32, isOutput=True)

    with tile.TileContext(nc) as tc:
        with (
            tc.tile_pool(name="const", bufs=1) as const,
            tc.tile_pool(name="otstream", bufs=6) as otstream,
            tc.tile_pool(name="mtstream", bufs=4) as mtstream,
            tc.tile_pool(name="vstream", bufs=2) as vstream,
            tc.tile_pool(name="mstream", bufs=2) as mstream,
            tc.tile_pool(name="ptiles", bufs=3) as ptiles,
            tc.tile_pool(name="pmtiles", bufs=3) as pmtiles,
            tc.tile_pool(name="otiles", bufs=4) as otiles,
            tc.tile_pool(name="small", bufs=2) as small,
            tc.tile_pool(name="ps_s", bufs=3, space="PSUM") as ps_s,
            tc.tile_pool(name="ps_av", bufs=1, space="PSUM") as ps_av,
            tc.tile_pool(name="ps_sum", bufs=1, space="PSUM") as ps_sum,
        ):
            # ---- persistent SBUF ----
            wq_sb = const.tile([P, N_JT, MID], bf)    # WqT j-tiles
            wk_sb = const.tile([P, N_JT, MID], bf)
            bq_sb = const.tile([P, N_DT], f32)
            bk_sb = const.tile([P, N_DT], f32)
            fix_sb = const.tile([P, N_MT], f32)
            ones_sb = const.tile([P, 32], bf)
            ident = const.tile([32, 32], f32)
            kt_sb = const.tile([P, N_DT, M], bf)      # K^T, d-tiles x m
            qt_sb = const.tile([P, N_DT, NSH], bf)    # Q^T, d-tiles x q
            v_sb = const.tile([P, N_MT, KDIM], bf)    # V m-tiles

            nc.sync.dma_start(out=wq_sb[:], in_=wqT.rearrange("(j p) d -> p j d", p=P))
            nc.sync.dma_start(out=wk_sb[:], in_=wkT.rearrange("(j p) d -> p j d", p=P))
            nc.sync.dma_start(out=bq_sb[:], in_=bq_t[:])
            nc.sync.dma_start(out=bk_sb[:], in_=bk_t[:])
            nc.sync.dma_start(out=fix_sb[:], in_=fix_t[:])
            nc.vector.memset(ones_sb[:], 1.0)
            make_identity(nc, ident[:])

            # ---- V = fix * other, bf16, m on partitions (emitted per group,
            # interleaved into supertile 0 so DMA self-paces) ----
            def emit_vgroup(vg):
                o_t = vstream.tile([P, G, KDIM], bf, tag="vsrc", name="vsrc")
                nc.sync.dma_start(
                    out=o_t[:],
                    in_=other[vg * G * P:(vg + 1) * G * P, :].rearrange(
                        "(s p) k -> p s k", p=P),
                )
                for s in range(G):
                    mt = vg * G + s
                    nc.vector.tensor_scalar_mul(
                        v_sb[:, mt, :], o_t[:, s, :], fix_sb[:, mt:mt + 1]
                    )

            # ---- Q^T = Wq @ main^T + bq ----
            mt_tiles = []
            for j in range(N_JT):
                t = mtstream.tile([P, NSH], bf, tag="mainT")
                nc.sync.dma_start(out=t[:], in_=mainT[j * P:(j + 1) * P, :])
                mt_tiles.append(t)
            for d in range(N_DT):
                for qc in range(NSH // 512):
                    ps = ps_s.tile([P, 512], f32, tag="s")
                    for j in range(N_JT):
                        nc.tensor.matmul(
                            ps[:],
                            wq_sb[:, j, d * P:(d + 1) * P],
                            mt_tiles[j][:, qc * 512:(qc + 1) * 512],
                            start=(j == 0),
                            stop=(j == N_JT - 1),
                        )
                    nc.scalar.activation(
                        qt_sb[:, d, qc * 512:(qc + 1) * 512], ps[:],
                        mybir.ActivationFunctionType.Identity,
                        bias=bq_sb[:, d:d + 1],
                    )

            # ---- fused K-proj + main attention loop ----
            # K-proj for a 2048-wide m range is emitted just before the
            # supertile-0 groups that consume it, so the (DMA-bound)
            # projection overlaps the (PE-bound) attention math.  Within a
            # supertile the m-loop is software-pipelined: S-matmuls run one
            # m-tile ahead of the dependent sums/AV matmuls so the
            # S->exp->mask chain latency hides behind PE work.
            MCG = 2048                                # m per prep group

            def emit_kproj(mg):
                ot_tiles = []
                for j in range(N_JT):
                    t = otstream.tile([P, MCG], bf, tag="otherT")
                    nc.sync.dma_start(
                        out=t[:],
                        in_=otherT[j * P:(j + 1) * P, mg * MCG:(mg + 1) * MCG],
                    )
                    ot_tiles.append(t)
                for d in range(N_DT):
                    for mc in range(MCG // 512):
                        ps = ps_s.tile([P, 512], f32, tag="s")
                        for j in range(N_JT):
                            nc.tensor.matmul(
                                ps[:],
                                wk_sb[:, j, d * P:(d + 1) * P],
                                ot_tiles[j][:, mc * 512:(mc + 1) * 512],
                                start=(j == 0),
                                stop=(j == N_JT - 1),
                            )
                        m0 = mg * MCG + mc * 512
                        nc.scalar.activation(
                            kt_sb[:, d, m0:m0 + 512], ps[:],
                            mybir.ActivationFunctionType.Identity,
                            bias=bk_sb[:, d:d + 1],
                        )

            st_state = {}

            def emit_group_front(st, g):
                # S matmuls + exp + mask for group g of supertile st
                q0 = st * 512
                nm_g = mstream.tile([P, G, 512], bf, tag="nm")
                nc.gpsimd.dma_start(
                    out=nm_g[:],
                    in_=nmaskT[g * G * P:(g + 1) * G * P, q0:q0 + 512]
                    .rearrange("(s p) q -> p s q", p=P),
                )
                p_g = ptiles.tile([P, G, 512], bf, tag="p")
                pm_g = pmtiles.tile([P, G, 512], bf, tag="pm")
                return nm_g, p_g, pm_g

            def emit_s_exp(st, g, s, nm_g, p_g, pm_g):
                q0 = st * 512
                mt = g * G + s
                s_ps = ps_s.tile([P, 512], f32, tag="s")
                for d in range(N_DT):
                    nc.tensor.matmul(
                        s_ps[:],
                        kt_sb[:, d, mt * P:(mt + 1) * P],
                        qt_sb[:, d, q0:q0 + 512],
                        start=(d == 0),
                        stop=(d == N_DT - 1),
                    )
                nc.scalar.activation(
                    p_g[:, s, :], s_ps[:],
                    mybir.ActivationFunctionType.Exp,
                    scale=1.0 / SCALE,
                )
                nc.vector.tensor_mul(
                    pm_g[:, s, :], p_g[:, s, :], nm_g[:, s, :]
                )

            def emit_back_group(st, g, pm_g):
                # same-bank runs: all 4 m-tiles of the group into one qs bank
                # before switching banks; sums last.
                av, sums = st_state[st]
                for qs in range(4):
                    for s in range(G):
                        mt = g * G + s
                        nc.tensor.matmul(
                            av[:, qs, :],
                            pm_g[:, s, qs * P:(qs + 1) * P],
                            v_sb[:, mt, :],
                            start=(mt == 0),
                            stop=(mt == N_MT - 1),
                        )
                for s in range(G):
                    mt = g * G + s
                    nc.tensor.matmul(
                        sums[:], ones_sb[:], pm_g[:, s, :],
                        start=(mt == 0), stop=(mt == N_MT - 1),
                    )

            def emit_tail(st):
                av, sums = st_state[st]
                q0 = st * 512
                sums_sb = small.tile([32, 512], f32, tag="sums_sb")
                nc.scalar.copy(sums_sb[:], sums[:])
                tr = ps_s.tile([P, 4, 32], f32, tag="s")
                for qs in range(4):
                    nc.tensor.transpose(
                        tr[:, qs, :], sums_sb[:, qs * P:(qs + 1) * P], ident[:]
                    )
                recip = small.tile([P, 4], f32, tag="recip")
                nc.vector.reciprocal(recip[:], tr[:, :, 0])
                for qs in range(4):
                    o_sb = otiles.tile([P, KDIM], f32, tag="o")
                    nc.vector.tensor_scalar_mul(
                        o_sb[:], av[:, qs, :], recip[:, qs:qs + 1]
                    )
                    nc.sync.dma_start(
                        out=out[q0 + qs * P:q0 + (qs + 1) * P, :],
                        in_=o_sb[:],
                    )

            def emit_supertile(st, interleave_kproj):
                st_state[st] = (
                    ps_av.tile([P, 4, KDIM], f32, tag="av", name="av"),
                    ps_sum.tile([32, 512], f32, tag="sums", name="sums"),
                )
                pending = None   # (g, pm_g): back-work lags one group
                for g in range(N_G):
                    if interleave_kproj and g % 4 == 0:
                        emit_kproj(g // 4)
                    if interleave_kproj:
                        emit_vgroup(g)
                    nm_g, p_g, pm_g = emit_group_front(st, g)
                    for s in range(G):
                        mt = g * G + s
                        emit_s_exp(st, g, s, nm_g, p_g, pm_g)
                    if pending is not None:
                        emit_back_group(st, *pending)
                    pending = (g, pm_g)
                if pending is not None:
                    emit_back_group(st, *pending)
                emit_tail(st)

            emit_supertile(0, True)
            emit_supertile(1, False)

    nc.compile()
    return nc


def _get_nc():
    global _BUILT
    if _BUILT is None:
        _BUILT = build_nc()
    return _BUILT


def kernel(main_feat, other_feat, fix_feat, mask, Wq, bq, Wk, bk):
    main_feat = np.asarray(main_feat, dtype=np.float32)
    other_feat = np.asarray(other_feat, dtype=np.float32)
    fix_feat = np.asarray(fix_feat, dtype=np.float32)
    mask = np.asarray(mask)
    Wq = np.asarray(Wq, dtype=np.float32)
    bq = np.asarray(bq, dtype=np.float32)
    Wk = np.asarray(Wk, dtype=np.float32)
    bk = np.asarray(bk, dtype=np.float32)

    nc = _get_nc()

    otherT_bf = np.ascontiguousarray(other_feat.T).astype(BF16)
    other_bf = other_feat.astype(BF16)
    wqT_bf = np.ascontiguousarray(Wq.T).astype(BF16)
    wkT_bf = np.ascontiguousarray(Wk.T).astype(BF16)
    bq_t = np.ascontiguousarray(bq.reshape(N_DT, P).T)
    bk_t = np.ascontiguousarray(bk.reshape(N_DT, P).T)
    fix_t = np.ascontiguousarray(fix_feat.reshape(N_MT, P).T)
    # notmask, transposed: [M, N] in bf16 (exact 0.0 / 1.0)
    nmaskT_bf = np.ascontiguousarray((~mask).T).astype(BF16)

    in_maps = []
    for c in range(NCORES):
        r0, r1 = c * NSH, (c + 1) * NSH
        in_maps.append({
            "mainT": np.ascontiguousarray(main_feat[r0:r1].T).astype(BF16),
            "otherT": otherT_bf,
            "other": other_bf,
            "nmaskT": np.ascontiguousarray(nmaskT_bf[:, r0:r1]),
            "wqT": wqT_bf,
            "wkT": wkT_bf,
            "bq_t": bq_t,
            "bk_t": bk_t,
            "fix_t": fix_t,
        })

    res = run_bass_kernel_spmd(nc, in_maps, list(range(NCORES)))
    return np.concatenate([res.results[c]["out"] for c in range(NCORES)], axis=0)


# revision 24
# speedup vs baseline: 1.1613x; 1.1613x over previous
"""Trainium2 Bass kernel for nn_AdjAttenAggr (masked attention aggregation).

Math (reference):
    Q = main_feat @ Wq.T + bq                 [N, MID]
    K = other_feat @ Wk.T + bk                [M, MID]
    A = softmax(where(mask, -2^32, Q K^T / sqrt(MID)), axis=-1)
    out = A @ (fix_feat[:, None] * other_feat)   [N, KDIM]

Strategy: row-parallel across 8 NeuronCores (1024 rows of N each), K/V
replicated.  On each core, flash-style: S^T tiles [m=128, q] from PE
(bf16 operands, fp32 PSUM), exp on ACT (scale folded in; scores are
small so no max subtraction is needed), multiplicative mask on DVE
(exp(s+NEG*mask) == exp(s)*(1-mask) since exp(-2^32) == 0), then
P^T @ V on PE accumulating over m.  Row sums ride along as a
ones-stationary matmul; normalization happens after the AV matmul since
softmax(S) @ V == diag(1/rowsum) @ (exp(S) @ V).
"""

import numpy as np
import ml_dtypes

import concourse.bacc as bacc
import concourse.tile as tile
from concourse import mybir
from concourse.bass_utils import run_bass_kernel_spmd
from concourse.masks import make_identity

BF16 = ml_dtypes.bfloat16

N, M = 8192, 8192
QDIM, KDIM, MID = 512, 512, 256
SCALE = float(np.sqrt(MID))
NCORES = 8
NSH = N // NCORES          # 1024 rows per core
P = 128                    # partitions
N_MT = M // P              # 64 m-tiles
N_ST = NSH // 512          # 2 supertiles of 512 q rows
N_JT = QDIM // P           # 4 contraction tiles for projections
N_DT = MID // P            # 2 d-tiles
G = 4                      # m-tiles per group
N_G = N_MT // G            # 16 groups

_BUILT = None


def build_nc():
    nc = bacc.Bacc(None, target_bir_lowering=False, debug=False)
    f32 = mybir.dt.float32
    bf = mybir.dt.bfloat16

    mainT = nc.declare_dram_parameter("mainT", [QDIM, NSH], bf, isOutput=False)
    otherT = nc.declare_dram_parameter("otherT", [KDIM, M], bf, isOutput=False)
    other = nc.declare_dram_parameter("other", [M, KDIM], bf, isOutput=False)
    nmaskT = nc.declare_dram_parameter("nmaskT", [M, NSH], bf, isOutput=False)
    wblob = nc.declare_dram_parameter("wblob", [P, 2 * N_JT * MID], bf, isOutput=False)
    fblob = nc.declare_dram_parameter("fblob", [P, 2 * N_DT + N_MT], f32, isOutput=False)
    out = nc.declare_dram_parameter("out", [NSH, KDIM], f32, isOutput=True)

    with tile.TileContext(nc) as tc:
        with (
            tc.tile_pool(name="const", bufs=1) as const,
            tc.tile_pool(name="otstream", bufs=6) as otstream,
            tc.tile_pool(name="mtstream", bufs=4) as mtstream,
            tc.tile_pool(name="vstream", bufs=2) as vstream,
            tc.tile_pool(name="mstream", bufs=3) as mstream,
            tc.tile_pool(name="ptiles", bufs=3) as ptiles,
            tc.tile_pool(name="pmtiles", bufs=3) as pmtiles,
            tc.tile_pool(name="otiles", bufs=4) as otiles,
            tc.tile_pool(name="small", bufs=2) as small,
            tc.tile_pool(name="ps_s", bufs=3, space="PSUM") as ps_s,
            tc.tile_pool(name="ps_av", bufs=1, space="PSUM") as ps_av,
            tc.tile_pool(name="ps_sum", bufs=1, space="PSUM") as ps_sum,
        ):
            # ---- persistent SBUF ----
            wq_sb = const.tile([P, N_JT, MID], bf)    # WqT j-tiles
            wk_sb = const.tile([P, N_JT, MID], bf)
            bq_sb = const.tile([P, N_DT], f32)
            bk_sb = const.tile([P, N_DT], f32)
            fix_sb = const.tile([P, N_MT], f32)
            ones_sb = const.tile([P, 32], bf)
            ident = const.tile([32, 32], f32)
            kt_sb = const.tile([P, N_DT, M], bf)      # K^T, d-tiles x m
            qt_sb = const.tile([P, N_DT, NSH], bf)    # Q^T, d-tiles x q
            v_sb = const.tile([P, N_MT, KDIM], bf)    # V m-tiles

            nc.sync.dma_start(
                out=wq_sb[:].rearrange("p j d -> p (j d)"),
                in_=wblob[:, :N_JT * MID])
            nc.sync.dma_start(
                out=wk_sb[:].rearrange("p j d -> p (j d)"),
                in_=wblob[:, N_JT * MID:])
            nc.sync.dma_start(out=bq_sb[:], in_=fblob[:, 0:N_DT])
            nc.sync.dma_start(out=bk_sb[:], in_=fblob[:, N_DT:2 * N_DT])
            nc.sync.dma_start(out=fix_sb[:], in_=fblob[:, 2 * N_DT:])
            nc.vector.memset(ones_sb[:], 1.0)
            make_identity(nc, ident[:])

            # ---- V = fix * other, bf16, m on partitions (emitted per group,
            # interleaved into supertile 0 so DMA self-paces) ----
            def emit_vgroup(vg):
                o_t = vstream.tile([P, G, KDIM], bf, tag="vsrc", name="vsrc")
                nc.sync.dma_start(
                    out=o_t[:],
                    in_=other[vg * G * P:(vg + 1) * G * P, :].rearrange(
                        "(s p) k -> p s k", p=P),
                )
                for s in range(G):
                    mt = vg * G + s
                    nc.vector.tensor_scalar_mul(
                        v_sb[:, mt, :], o_t[:, s, :], fix_sb[:, mt:mt + 1]
                    )

            # ---- Q^T = Wq @ main^T + bq ----
            mt_tiles = []
            for j in range(N_JT):
                t = mtstream.tile([P, NSH], bf, tag="mainT")
                nc.sync.dma_start(out=t[:], in_=mainT[j * P:(j + 1) * P, :])
                mt_tiles.append(t)
            for d in range(N_DT):
                for qc in range(NSH // 512):
                    ps = ps_s.tile([P, 512], f32, tag="s")
                    for j in range(N_JT):
                        nc.tensor.matmul(
                            ps[:],
                            wq_sb[:, j, d * P:(d + 1) * P],
                            mt_tiles[j][:, qc * 512:(qc + 1) * 512],
                            start=(j == 0),
                            stop=(j == N_JT - 1),
                        )
                    nc.scalar.activation(
                        qt_sb[:, d, qc * 512:(qc + 1) * 512], ps[:],
                        mybir.ActivationFunctionType.Identity,
                        bias=bq_sb[:, d:d + 1],
                    )

            # ---- fused K-proj + main attention loop ----
            # K-proj for a 2048-wide m range is emitted just before the
            # supertile-0 groups that consume it, so the (DMA-bound)
            # projection overlaps the (PE-bound) attention math.  Within a
            # supertile the m-loop is software-pipelined: S-matmuls run one
            # m-tile ahead of the dependent sums/AV matmuls so the
            # S->exp->mask chain latency hides behind PE work.
            MCG = 2048                                # m per prep group

            def emit_kproj(mg):
                ot_tiles = []
                for j in range(N_JT):
                    t = otstream.tile([P, MCG], bf, tag="otherT")
                    nc.sync.dma_start(
                        out=t[:],
                        in_=otherT[j * P:(j + 1) * P, mg * MCG:(mg + 1) * MCG],
                    )
                    ot_tiles.append(t)
                for d in range(N_DT):
                    for mc in range(MCG // 512):
                        ps = ps_s.tile([P, 512], f32, tag="s")
                        for j in range(N_JT):
                            nc.tensor.matmul(
                                ps[:],
                                wk_sb[:, j, d * P:(d + 1) * P],
                                ot_tiles[j][:, mc * 512:(mc + 1) * 512],
                                start=(j == 0),
                                stop=(j == N_JT - 1),
                            )
                        m0 = mg * MCG + mc * 512
                        nc.scalar.activation(
                            kt_sb[:, d, m0:m0 + 512], ps[:],
                            mybir.ActivationFunctionType.Identity,
                            bias=bk_sb[:, d:d + 1],
                        )

            st_state = {}

            def emit_group_front(st, g):
                # S matmuls + exp + mask for group g of supertile st
                q0 = st * 512
                nm_g = mstream.tile([P, G, 512], bf, tag="nm")
                nc.gpsimd.dma_start(
                    out=nm_g[:],
                    in_=nmaskT[g * G * P:(g + 1) * G * P, q0:q0 + 512]
                    .rearrange("(s p) q -> p s q", p=P),
                )
                p_g = ptiles.tile([P, G, 512], bf, tag="p")
                pm_g = pmtiles.tile([P, G, 512], bf, tag="pm")
                return nm_g, p_g, pm_g

            def emit_s_exp(st, g, s, nm_g, p_g, pm_g):
                q0 = st * 512
                mt = g * G + s
                s_ps = ps_s.tile([P, 512], f32, tag="s")
                for d in range(N_DT):
                    nc.tensor.matmul(
                        s_ps[:],
                        kt_sb[:, d, mt * P:(mt + 1) * P],
                        qt_sb[:, d, q0:q0 + 512],
                        start=(d == 0),
                        stop=(d == N_DT - 1),
                    )
                nc.scalar.activation(
                    p_g[:, s, :], s_ps[:],
                    mybir.ActivationFunctionType.Exp,
                    scale=1.0 / SCALE,
                )
                nc.vector.tensor_mul(
                    pm_g[:, s, :], p_g[:, s, :], nm_g[:, s, :]
                )

            def emit_back_group(st, g, pm_g):
                # same-bank runs: all 4 m-tiles of the group into one qs bank
                # before switching banks; sums last.
                av, sums = st_state[st]
                for qs in range(4):
                    for s in range(G):
                        mt = g * G + s
                        nc.tensor.matmul(
                            av[:, qs, :],
                            pm_g[:, s, qs * P:(qs + 1) * P],
                            v_sb[:, mt, :],
                            start=(mt == 0),
                            stop=(mt == N_MT - 1),
                        )
                for s in range(G):
                    mt = g * G + s
                    nc.tensor.matmul(
                        sums[:], ones_sb[:], pm_g[:, s, :],
                        start=(mt == 0), stop=(mt == N_MT - 1),
                    )

            def emit_tail(st):
                av, sums = st_state[st]
                q0 = st * 512
                sums_sb = small.tile([32, 512], f32, tag="sums_sb")
                nc.scalar.copy(sums_sb[:], sums[:])
                tr = ps_s.tile([P, 4, 32], f32, tag="s")
                for qs in range(4):
                    nc.tensor.transpose(
                        tr[:, qs, :], sums_sb[:, qs * P:(qs + 1) * P], ident[:]
                    )
                recip = small.tile([P, 4], f32, tag="recip")
                nc.vector.reciprocal(recip[:], tr[:, :, 0])
                for qs in range(4):
                    o_sb = otiles.tile([P, KDIM], f32, tag="o")
                    nc.vector.tensor_scalar_mul(
                        o_sb[:], av[:, qs, :], recip[:, qs:qs + 1]
                    )
                    nc.sync.dma_start(
                        out=out[q0 + qs * P:q0 + (qs + 1) * P, :],
                        in_=o_sb[:],
                    )

            def emit_supertile(st, interleave_kproj):
                st_state[st] = (
                    ps_av.tile([P, 4, KDIM], f32, tag="av", name="av"),
                    ps_sum.tile([32, 512], f32, tag="sums", name="sums"),
                )
                pending = None   # (g, pm_g): back-work lags one group
                for g in range(N_G):
                    if interleave_kproj and g % 4 == 0:
                        emit_kproj(g // 4)
                    if interleave_kproj:
                        emit_vgroup(g)
                    nm_g, p_g, pm_g = emit_group_front(st, g)
                    for s in range(G):
                        mt = g * G + s
                        emit_s_exp(st, g, s, nm_g, p_g, pm_g)
                    if pending is not None:
                        emit_back_group(st, *pending)
                    pending = (g, pm_g)
                if pending is not None:
                    emit_back_group(st, *pending)
                emit_tail(st)

            emit_supertile(0, True)
            emit_supertile(1, False)

    nc.compile()
    return nc


def _get_nc():
    global _BUILT
    if _BUILT is None:
        _BUILT = build_nc()
    return _BUILT


def kernel(main_feat, other_feat, fix_feat, mask, Wq, bq, Wk, bk):
    main_feat = np.asarray(main_feat, dtype=np.float32)
    other_feat = np.asarray(other_feat, dtype=np.float32)
    fix_feat = np.asarray(fix_feat, dtype=np.float32)
    mask = np.asarray(mask)
    Wq = np.asarray(Wq, dtype=np.float32)
    bq = np.asarray(bq, dtype=np.float32)
    Wk = np.asarray(Wk, dtype=np.float32)
    bk = np.asarray(bk, dtype=np.float32)

    nc = _get_nc()

    otherT_bf = np.ascontiguousarray(other_feat.T).astype(BF16)
    other_bf = other_feat.astype(BF16)
    wq_p = Wq.T.reshape(N_JT, P, MID).transpose(1, 0, 2).reshape(P, N_JT * MID)
    wk_p = Wk.T.reshape(N_JT, P, MID).transpose(1, 0, 2).reshape(P, N_JT * MID)
    wblob = np.ascontiguousarray(np.concatenate([wq_p, wk_p], axis=1)).astype(BF16)
    fblob = np.ascontiguousarray(np.concatenate([
        bq.reshape(N_DT, P).T, bk.reshape(N_DT, P).T,
        fix_feat.reshape(N_MT, P).T], axis=1))
    # notmask, transposed: [M, N] in bf16 (exact 0.0 / 1.0)
    nmaskT_bf = np.ascontiguousarray((~mask).T).astype(BF16)

    in_maps = []
    for c in range(NCORES):
        r0, r1 = c * NSH, (c + 1) * NSH
        in_maps.append({
            "mainT": np.ascontiguousarray(main_feat[r0:r1].T).astype(BF16),
            "otherT": otherT_bf,
            "other": other_bf,
            "nmaskT": np.ascontiguousarray(nmaskT_bf[:, r0:r1]),
            "wblob": wblob,
            "fblob": fblob,
        })

    res = run_bass_kernel_spmd(nc, in_maps, list(range(NCORES)))
    return np.concatenate([res.results[c]["out"] for c in range(NCORES)], axis=0)
